# revision 32
# baseline (speedup 1.0000x reference)
"""MinLSTM Trainium2 kernel.

Math (identical to the log-space reference, in linear space):
    sf = sigmoid(x @ W_f.T + b_f)
    si = sigmoid(x @ W_i.T + b_i)
    zh = x @ W_h.T + b_h
    g  = where(zh >= 0, zh + 0.5, sigmoid(zh)) = max(zh + 0.5, sigmoid(zh))
    q  = si / (sf + si)          (normalized input gate)
    a  = 1 - q                   (normalized forget gate)
    b  = q * g
    h_t = a_t * h_{t-1} + b_t    (hardware tensor_tensor_scan, fp32 state)

Sharding: data-parallel over batch B=8, one batch per NeuronCore.

Precision design (validated vs the f32 jax reference, gate 2e-2; this
config measures 1.1e-2):
  - x is quantized once to fp8 e4m3 (values ~N(0,1), fits the normal
    range) and is the single moving operand for all three gate matmuls.
  - f/i gate weights: fp8 e4m3 DoubleRow (2 k-blocks per matmul),
    pre-scaled by 32 into e4m3's normal range; the 1/32 is folded into
    the ACT sigmoid's input scale. Gate noise from fp8 z's is suppressed
    by sigmoid' ~ (1-a), so long-memory channels don't amplify it.
  - h gate weights: bf16 (mixed fp8-x * bf16-W matmul). zh noise enters
    h linearly (unsuppressed), so W_h gets the accurate path.
  - elementwise + scan operands + output: fp16. bf16 here measurably
    hurts (forget-gate quantization is amplified by long-memory
    channels); fp16's 2^-11 steps are accuracy-free while keeping
    2-byte DVE speed modes. Scan state is fp32 inside the hardware op.

Performance notes (measured on HW, loop-calibrated):
  - Engines contend on shared SBUF/fetch bandwidth: N concurrent engines
    deliver only ~1.7x one engine's throughput, so total work across
    engines is what matters, not per-engine balance alone.
  - PE is moving-operand-bandwidth bound at ~1 column/cycle regardless
    of dtype; DoubleRow only halves instruction count. Pure-DoubleRow
    streams stall on their 256-row weight loads; interleaving the h
    gate's plain matmuls between DoubleRow groups hides them (mix is
    faster than all-DR or all-plain).
  - The per-tile pipeline: PE -> ACT (3 sigmoids, PSUM-freeing) ->
    DVE (gg via scalar_tensor_tensor, recip, 1-q, b, 2x512-chunk scans)
    with den/q multiplies on gpsimd, two-stage software pipelined.
"""

import os
import sys

for _p in ("/opt/trn_rl_repo", "/root/.axon_site/_ro/trn_rl_repo"):
    if os.path.isdir(_p) and _p not in sys.path:
        sys.path.insert(0, _p)

import ml_dtypes
import numpy as np

import concourse.bacc as bacc
import concourse.tile as tile
from concourse import bass_utils, mybir
from concourse.mybir import ActivationFunctionType as AF
from concourse.mybir import AluOpType as ALU
from concourse.mybir import MatmulPerfMode as MM

B, T, D, H = 8, 4096, 512, 512
P = 128
KD = D // P       # 4 contraction blocks
KP = KD // 2      # 2 DoubleRow k-pair blocks
HB = H // P       # 4 hidden-partition blocks
W8SCALE = 32.0    # fp8 weight pre-scale (power of 2)
F32 = mybir.dt.float32
BF16 = mybir.dt.bfloat16
FP16 = mybir.dt.float16
FP8 = mybir.dt.float8e4

_CACHE = {}


def _build(n_cores=B, reps=1, loop_reps=0, tte=1024, mm_mode="fp8x",
           gg_mode="stt", den_eng="dma", qq_eng="gpsimd", scan_chunk=512,
           ew_dt=FP16, out_dt=FP16,
           sb_bufs=4, hop_bufs=8, loads_in_loop=True, ablate=()):
    nc = bacc.Bacc("TRN2", target_bir_lowering=False, debug=False,
                   num_devices=n_cores)
    fp8_gates = 2 if mm_mode in ("mix", "fp8x") else \
        (3 if mm_mode == "fp8" else 0)
    bf_gates = 3 - fp8_gates
    use_xb = bf_gates and mm_mode != "fp8x"
    # fp8 x: [KP][128, 2, T]; bf16 x: [KD][128, T]
    xq_d = (nc.dram_tensor("xq", [KP, P, 2, T], FP8, kind="ExternalInput")
            if fp8_gates else None)
    xb_d = (nc.dram_tensor("xb", [KD, P, T], BF16, kind="ExternalInput")
            if use_xb else None)
    wq_d = (nc.dram_tensor("wq", [fp8_gates, KP, P, 2, H], FP8,
                           kind="ExternalInput") if fp8_gates else None)
    wb_d = (nc.dram_tensor("wb", [bf_gates, KD, P, H], BF16,
                           kind="ExternalInput") if bf_gates else None)
    # 6 bias groups packed per partition plus a -0.5 constant column:
    # [b_f | b_i | b_h | b_h + 0.5 | -b_f | -b_i | -0.5]
    bias_d = nc.dram_tensor("biasp", [P, 6 * HB + 1], F32,
                            kind="ExternalInput")
    h0_d = nc.dram_tensor("h0p", [P, HB], F32, kind="ExternalInput")
    ht_d = nc.dram_tensor("ht", [H, T], out_dt, kind="ExternalOutput")

    NTE = T // tte
    NH = tte // 512   # matmul N=512 groups per tile
    n_zbufs = 8 * 512 // tte  # PSUM: 8 banks, each z tile is tte/512 banks

    with tile.TileContext(nc) as tc:
        with (
            tc.tile_pool(name="xp", bufs=1) as xp,
            tc.tile_pool(name="wp", bufs=1) as wp,
            tc.tile_pool(name="cp", bufs=1) as cp,
            tc.tile_pool(name="ps", bufs=n_zbufs, space="PSUM") as ps,
            tc.tile_pool(name="sb", bufs=sb_bufs) as sb,
            tc.tile_pool(name="hop", bufs=hop_bufs) as hop,
        ):
            # tiny constants first, then prime the ACT sigmoid table so the
            # ~1.3us table load is off the critical path
            bias = cp.tile([P, 6 * HB + 1], F32, tag="bias")
            nc.sync.dma_start(bias[:], bias_d.ap())
            h0 = cp.tile([P, HB], F32, tag="h0")
            nc.sync.dma_start(h0[:], h0_d.ap())
            warm = cp.tile([P, 1], F32, tag="warm")
            nc.scalar.activation(warm[:], h0[:, 0:1], AF.Sigmoid)

            xq = [xp.tile([P, 2, T], FP8, tag=f"xq{p}", name=f"xq{p}")
                  for p in range(KP)] if fp8_gates else []
            xb = [xp.tile([P, T], BF16, tag=f"xb{k}", name=f"xb{k}")
                  for k in range(KD)] if use_xb else []
            wq = [[wp.tile([P, 2, H], FP8, tag=f"wq{g}{p}", name=f"wq{g}{p}")
                   for p in range(KP)] for g in range(fp8_gates)]
            wb = [[wp.tile([P, H], BF16, tag=f"wb{g}{k}", name=f"wb{g}{k}")
                   for k in range(KD)] for g in range(bf_gates)]

            def load_w():
                for g in range(fp8_gates):
                    for p in range(KP):
                        nc.sync.dma_start(wq[g][p][:], wq_d.ap()[g, p])
                for g in range(bf_gates):
                    for k in range(KD):
                        nc.sync.dma_start(wb[g][k][:], wb_d.ap()[g, k])

            def load_x(tt):
                tsl = slice(tt * tte, (tt + 1) * tte)
                if fp8_gates:
                    for p in range(KP):
                        nc.sync.dma_start(xq[p][:, :, tsl],
                                          xq_d.ap()[p][:, :, tsl])
                if use_xb:
                    for k in range(KD):
                        nc.sync.dma_start(xb[k][:, tsl], xb_d.ap()[k][:, tsl])

            load_w()
            if not (loop_reps and loads_in_loop):
                for tt in range(NTE):
                    load_x(tt)

            import contextlib
            loop_cm = (tc.For_i(0, loop_reps, 1) if loop_reps
                       else contextlib.nullcontext())
            with loop_cm:
             for _rep in range(reps):
              if loop_reps and loads_in_loop:
                  for tt in range(NTE):
                      load_x(tt)
              prev = [None] * HB
              pend = None

              def front(tt, hb):
                    tsl = slice(tt * tte, (tt + 1) * tte)
                    hsl = slice(hb * P, (hb + 1) * P)
                    bfv = bias[:, 0 * HB + hb:0 * HB + hb + 1]
                    biv = bias[:, 1 * HB + hb:1 * HB + hb + 1]
                    bh05 = bias[:, 3 * HB + hb:3 * HB + hb + 1]
                    bhv = bias[:, 2 * HB + hb:2 * HB + hb + 1]
                    zf = ps.tile([P, tte], F32, tag="z")
                    zi = ps.tile([P, tte], F32, tag="z")
                    zh = ps.tile([P, tte], F32, tag="z")
                    zs = (zf, zi, zh)
                    if "mm" not in ablate:
                        # Interleave the h gate's plain matmuls between the
                        # f/i DoubleRow groups: a DR matmul's 256-row weight
                        # load hides behind the preceding plain matmul's
                        # stream (pure-DR streams stall on ldweights).
                        def h_mms(nh, ks):
                            z = zs[fp8_gates]
                            c0 = tt * tte + nh * 512
                            for k in ks:
                                rhs = (xb[k][:, c0:c0 + 512] if use_xb
                                       else xq[k // 2]
                                       [:, k % 2:k % 2 + 1, c0:c0 + 512])
                                nc.tensor.matmul(
                                    z[:, nh * 512:(nh + 1) * 512],
                                    wb[0][k][:, hsl],
                                    rhs,
                                    start=(k == 0), stop=(k == KD - 1),
                                )

                        def dr_mms(g, nh):
                            z = zs[g]
                            c0 = tt * tte + nh * 512
                            for p in range(KP):
                                nc.tensor.matmul(
                                    z[:, nh * 512:(nh + 1) * 512],
                                    wq[g][p][:, :, hsl],
                                    xq[p][:, :, c0:c0 + 512],
                                    start=(p == 0), stop=(p == KP - 1),
                                    perf_mode=MM.DoubleRow,
                                )

                        def dr_one(g, nh, p):
                            z = zs[g]
                            c0 = tt * tte + nh * 512
                            nc.tensor.matmul(
                                z[:, nh * 512:(nh + 1) * 512],
                                wq[g][p][:, :, hsl],
                                xq[p][:, :, c0:c0 + 512],
                                start=(p == 0), stop=(p == KP - 1),
                                perf_mode=MM.DoubleRow,
                            )

                        if bf_gates:
                            # alternate plain h matmuls with single DR
                            # matmuls so every DR 256-row weight load hides
                            # behind ~1024 cycles of streaming
                            for nh in range(NH):
                                h_mms(nh, [0])
                                dr_one(0, nh, 0)
                                h_mms(nh, [1])
                                dr_one(0, nh, 1)
                                h_mms(nh, [2])
                                dr_one(1, nh, 0)
                                h_mms(nh, [3])
                                dr_one(1, nh, 1)
                        else:
                            for nh in range(NH):
                                for g in range(fp8_gates):
                                    dr_mms(g, nh)
                    fsc = 1.0 / W8SCALE if fp8_gates >= 1 else 1.0
                    isc = 1.0 / W8SCALE if fp8_gates >= 2 else 1.0
                    hsc = 1.0 / W8SCALE if fp8_gates >= 3 else 1.0
                    tf = sb.tile([P, tte], ew_dt, tag="tf")
                    nc.scalar.activation(tf[:], zf[:], AF.Sigmoid,
                                         bias=bfv, scale=fsc)
                    ti = sb.tile([P, tte], ew_dt, tag="ti")
                    nc.scalar.activation(ti[:], zi[:], AF.Sigmoid,
                                         bias=biv, scale=isc)
                    gg = sb.tile([P, tte], ew_dt, tag="gg")
                    if gg_mode == "lin":
                        # lin = zh + (b_h + 0.5) via ACT Identity consumes
                        # zh's PSUM immediately; th = sigmoid(lin - 0.5)
                        # reads SBUF instead of PSUM.
                        lin = sb.tile([P, tte], ew_dt, tag="lin")
                        nc.scalar.activation(lin[:], zh[:], AF.Identity,
                                             bias=bh05, scale=hsc)
                        th = sb.tile([P, tte], ew_dt, tag="th")
                        nc.scalar.activation(th[:], lin[:], AF.Sigmoid,
                                             bias=bias[:, 6 * HB:6 * HB + 1])
                        nc.vector.tensor_tensor(gg[:], lin[:], th[:], ALU.max)
                    else:
                        assert hsc == 1.0
                        th = sb.tile([P, tte], ew_dt, tag="th")
                        nc.scalar.activation(th[:], zh[:], AF.Sigmoid,
                                             bias=bhv)
                        nc.vector.scalar_tensor_tensor(
                            gg[:], zh[:], bh05, th[:], ALU.add, ALU.max)
                    den32 = sb.tile([P, tte], F32, tag="den32")
                    if den_eng == "dma":
                        # den = tf + ti on the DMA engines: gpsimd-initiated
                        # DMAs can cast (fp16->f32) and accumulate, freeing
                        # Pool/DVE cycles entirely.
                        nc.gpsimd.dma_start(den32[:], tf[:])
                        nc.gpsimd.dma_start(den32[:], ti[:],
                                            accum_op=ALU.add)
                    else:
                        getattr(nc, den_eng).tensor_tensor(den32[:], tf[:],
                                                           ti[:], ALU.add)
                    return dict(tt=tt, hb=hb, ti=ti, gg=gg, den32=den32)

              def back(st):
                    tt, hb = st["tt"], st["hb"]
                    tsl = slice(tt * tte, (tt + 1) * tte)
                    hsl = slice(hb * P, (hb + 1) * P)
                    rr = sb.tile([P, tte], F32, tag="rr")
                    nc.vector.reciprocal_approx_fast(rr[:], st["den32"][:])
                    qq = sb.tile([P, tte], ew_dt, tag="qq")
                    getattr(nc, qq_eng).tensor_tensor(qq[:], st["ti"][:],
                                                      rr[:], ALU.mult)
                    aa = sb.tile([P, tte], ew_dt, tag="aa")
                    nc.vector.tensor_scalar(aa[:], qq[:], -1.0, 1.0,
                                            ALU.mult, ALU.add)
                    bb = sb.tile([P, tte], ew_dt, tag="bb")
                    nc.vector.tensor_tensor(bb[:], qq[:], st["gg"][:],
                                            ALU.mult)
                    if "scan" not in ablate:
                        ho = hop.tile([P, tte], out_dt, tag="ho")
                        init = (h0[:, hb:hb + 1] if tt == 0
                                else prev[hb][:, tte - 1:tte])
                        for j0 in range(0, tte, scan_chunk):
                            jsl = slice(j0, j0 + scan_chunk)
                            nc.vector.tensor_tensor_scan(
                                ho[:, jsl], aa[:, jsl], bb[:, jsl], init,
                                ALU.mult, ALU.add)
                            init = ho[:, j0 + scan_chunk - 1:j0 + scan_chunk]
                        prev[hb] = ho
                    else:
                        ho = bb
                    if "dma" not in ablate:
                        nc.sync.dma_start(ht_d.ap()[hsl, tsl], ho[:])

              for tt in range(NTE):
                for hb in range(HB):
                    st = front(tt, hb)
                    if pend is not None:
                        back(pend)
                    pend = st
              back(pend)
            if "dma" in ablate or "act" in ablate:
                dummy = cp.tile([P, HB], out_dt, tag="dummy")
                nc.vector.memset(dummy[:], 0.0)
                nc.sync.dma_start(ht_d.ap()[0:P, 0:HB], dummy[:])

    nc.compile()
    return nc


def _get_module():
    if "nc" not in _CACHE:
        _CACHE["nc"] = _build()
    return _CACHE["nc"]


class _Runner:
    """Caches a compiled 8-core shard_map'd PJRT executable of the Bass
    module so repeat kernel() calls skip jax retracing/compilation."""

    def __init__(self, nc):
        import jax
        from jax.experimental.shard_map import shard_map
        from jax.sharding import Mesh, PartitionSpec

        from concourse import bass2jax

        bass2jax.install_neuronx_cc_hook()
        self.nc = nc
        partition_name = (nc.partition_id_tensor.name
                          if nc.partition_id_tensor else None)
        in_names, out_names, out_avals = [], [], []
        for alloc in nc.m.functions[0].allocations:
            if not isinstance(alloc, mybir.MemoryLocationSet):
                continue
            name = alloc.memorylocations[0].name
            if alloc.kind == "ExternalInput":
                if name != partition_name:
                    in_names.append(name)
            elif alloc.kind == "ExternalOutput":
                out_names.append(name)
                out_avals.append(jax.core.ShapedArray(
                    tuple(alloc.tensor_shape), mybir.dt.np(alloc.dtype)))
        self.in_names = in_names
        self.out_names = out_names
        self.out_avals = out_avals
        n_params, n_outs = len(in_names), len(out_names)
        all_names = list(in_names) + list(out_names)
        if partition_name is not None:
            all_names.append(partition_name)

        def _body(*args):
            operands = list(args)
            if partition_name is not None:
                operands.append(bass2jax.partition_id_tensor())
            return tuple(bass2jax._bass_exec_p.bind(
                *operands,
                out_avals=tuple(out_avals),
                in_names=tuple(all_names),
                out_names=tuple(out_names),
                lowering_input_output_aliases=(),
                sim_require_finite=True,
                sim_require_nnan=True,
                nc=nc,
            ))

        devices = jax.devices()[:B]
        mesh = Mesh(np.asarray(devices), ("core",))
        specs = (PartitionSpec("core"),) * (n_params + n_outs)
        out_specs = (PartitionSpec("core"),) * n_outs
        donate = tuple(range(n_params, n_params + n_outs))
        self._jitted = jax.jit(
            shard_map(_body, mesh=mesh, in_specs=specs,
                      out_specs=out_specs, check_rep=False),
            donate_argnums=donate, keep_unused=True)
        self._compiled = None

    def concat_args(self, in_maps):
        concat_in = [
            np.concatenate([np.asarray(m[name]) for m in in_maps], axis=0)
            for name in self.in_names
        ]
        concat_zeros = [
            np.zeros((B * a.shape[0], *a.shape[1:]), a.dtype)
            for a in self.out_avals
        ]
        return concat_in + concat_zeros

    def compiled(self, args):
        if self._compiled is None:
            self._compiled = self._jitted.lower(*args).compile()
        return self._compiled

    def __call__(self, in_maps):
        import jax
        args = self.concat_args(in_maps)
        outs = jax.block_until_ready(self.compiled(args)(*args))
        return [
            {name: np.asarray(outs[i]).reshape(B, *self.out_avals[i].shape)[c]
             for i, name in enumerate(self.out_names)}
            for c in range(B)
        ]


def _get_runner():
    if "runner" not in _CACHE:
        _CACHE["runner"] = _Runner(_get_module())
    return _CACHE["runner"]


def make_in_maps(x, h_0, W_f, b_f, W_i, b_i, W_h, b_h,
                 mm_mode="fp8x"):
    x = np.asarray(x, np.float32)
    h_0 = np.asarray(h_0, np.float32)
    Ws = [np.asarray(W_f, np.float32), np.asarray(W_i, np.float32),
          np.asarray(W_h, np.float32)]
    fp8_gates = 2 if mm_mode in ("mix", "fp8x") else \
        (3 if mm_mode == "fp8" else 0)
    bf_gates = 3 - fp8_gates
    use_xb = bf_gates and mm_mode != "fp8x"

    # fp8 weights: [g][kp][r, j, h] = 32*W_g[h, (2p+j)*128+r], stored
    # [fp8_gates, KP, P, 2*H]
    if fp8_gates:
        wq = np.empty((fp8_gates, KP, P, 2, H), dtype=ml_dtypes.float8_e4m3)
        for g in range(fp8_gates):
            Wt = (Ws[g] * W8SCALE).T.reshape(KP, 2, P, H)  # [kp, j, r, h]
            wq[g] = Wt.transpose(0, 2, 1, 3).astype(ml_dtypes.float8_e4m3)
        wq = np.ascontiguousarray(wq)
    # bf16 weights (the last bf_gates of [f,i,h]): [g][k][r, h] = W[h, k*128+r]
    if bf_gates:
        wb = np.empty((bf_gates, KD, P, H), dtype=ml_dtypes.bfloat16)
        for g in range(bf_gates):
            Wt = Ws[fp8_gates + g].T.reshape(KD, P, H)
            wb[g] = Wt.astype(ml_dtypes.bfloat16)
        wb = np.ascontiguousarray(wb)

    b_h = np.asarray(b_h, np.float32)
    b_f = np.asarray(b_f, np.float32)
    b_i = np.asarray(b_i, np.float32)
    biasp = (np.stack([b_f, b_i, b_h, b_h + 0.5, -b_f, -b_i])
             .astype(np.float32).reshape(6, HB, P).transpose(2, 0, 1)
             .reshape(P, 6 * HB))
    biasp = np.ascontiguousarray(
        np.concatenate([biasp, np.full((P, 1), -0.5, np.float32)], axis=1))

    in_maps = []
    for b in range(B):
        m = {"biasp": biasp,
             "h0p": np.ascontiguousarray(h_0[b].reshape(HB, P).T)}
        xT = x[b].T  # [D, T]
        if fp8_gates:
            xq = xT.reshape(KP, 2, P, T).transpose(0, 2, 1, 3)  # [kp, r, j, t]
            m["xq"] = np.ascontiguousarray(xq.astype(ml_dtypes.float8_e4m3))
            m["wq"] = wq
        if bf_gates:
            if use_xb:
                m["xb"] = np.ascontiguousarray(
                    xT.reshape(KD, P, T).astype(ml_dtypes.bfloat16))
            m["wb"] = wb
        in_maps.append(m)
    return in_maps


def kernel(x, h_0, W_f, b_f, W_i, b_i, W_h, b_h):
    in_maps = make_in_maps(x, h_0, W_f, b_f, W_i, b_i, W_h, b_h)
    results = _get_runner()(in_maps)
    out = np.empty((B, T, H), np.float32)
    for b in range(B):
        out[b] = results[b]["ht"].astype(np.float32).T
    return out


# revision 35
# speedup vs baseline: 1.1854x; 1.1854x over previous
"""MinLSTM Trainium2 kernel.

Math (identical to the log-space reference, in linear space):
    sf = sigmoid(x @ W_f.T + b_f)
    si = sigmoid(x @ W_i.T + b_i)
    zh = x @ W_h.T + b_h
    g  = where(zh >= 0, zh + 0.5, sigmoid(zh)) = max(zh + 0.5, sigmoid(zh))
    q  = si / (sf + si)          (normalized input gate)
    a  = 1 - q                   (normalized forget gate)
    b  = q * g
    h_t = a_t * h_{t-1} + b_t    (hardware tensor_tensor_scan, fp32 state)

Sharding: data-parallel over batch B=8, one batch per NeuronCore.

Precision design (validated vs the f32 jax reference, gate 2e-2; this
config measures 1.1e-2):
  - x is quantized once to fp8 e4m3 (values ~N(0,1), fits the normal
    range) and is the single moving operand for all three gate matmuls.
  - f/i gate weights: fp8 e4m3 DoubleRow (2 k-blocks per matmul),
    pre-scaled by 32 into e4m3's normal range; the 1/32 is folded into
    the ACT sigmoid's input scale. Gate noise from fp8 z's is suppressed
    by sigmoid' ~ (1-a), so long-memory channels don't amplify it.
  - h gate weights: bf16 (mixed fp8-x * bf16-W matmul). zh noise enters
    h linearly (unsuppressed), so W_h gets the accurate path.
  - elementwise + scan operands + output: fp16. bf16 here measurably
    hurts (forget-gate quantization is amplified by long-memory
    channels); fp16's 2^-11 steps are accuracy-free while keeping
    2-byte DVE speed modes. Scan state is fp32 inside the hardware op.

Performance notes (measured on HW, loop-calibrated):
  - Engines contend on shared SBUF/fetch bandwidth: N concurrent engines
    deliver only ~1.7x one engine's throughput, so total work across
    engines is what matters, not per-engine balance alone.
  - PE is moving-operand-bandwidth bound at ~1 column/cycle regardless
    of dtype; DoubleRow only halves instruction count. Pure-DoubleRow
    streams stall on their 256-row weight loads; interleaving the h
    gate's plain matmuls between DoubleRow groups hides them (mix is
    faster than all-DR or all-plain).
  - The per-tile pipeline: PE -> ACT (3 sigmoids, PSUM-freeing) ->
    DVE (gg via scalar_tensor_tensor, recip, 1-q, b, 2x512-chunk scans)
    with den/q multiplies on gpsimd, two-stage software pipelined.
"""

import os
import sys

for _p in ("/opt/trn_rl_repo", "/root/.axon_site/_ro/trn_rl_repo"):
    if os.path.isdir(_p) and _p not in sys.path:
        sys.path.insert(0, _p)

import ml_dtypes
import numpy as np

import concourse.bacc as bacc
import concourse.tile as tile
from concourse import bass_utils, mybir
from concourse.mybir import ActivationFunctionType as AF
from concourse.mybir import AluOpType as ALU
from concourse.mybir import MatmulPerfMode as MM

B, T, D, H = 8, 4096, 512, 512
P = 128
KD = D // P       # 4 contraction blocks
KP = KD // 2      # 2 DoubleRow k-pair blocks
HB = H // P       # 4 hidden-partition blocks
W8SCALE = 32.0    # fp8 weight pre-scale (power of 2)
F32 = mybir.dt.float32
BF16 = mybir.dt.bfloat16
FP16 = mybir.dt.float16
FP8 = mybir.dt.float8e4

_CACHE = {}


def _build(n_cores=B, reps=1, loop_reps=0, tte=1024, mm_mode="fp8x",
           gg_mode="stt", den_eng="dma", qq_eng="gpsimd", bb_eng="vector",
           scan_chunk=512,
           ew_dt=FP16, out_dt=FP16,
           sb_bufs=4, hop_bufs=8, loads_in_loop=True, ablate=()):
    nc = bacc.Bacc("TRN2", target_bir_lowering=False, debug=False,
                   num_devices=n_cores)
    fp8_gates = 2 if mm_mode in ("mix", "fp8x") else \
        (3 if mm_mode == "fp8" else 0)
    bf_gates = 3 - fp8_gates
    use_xb = bf_gates and mm_mode != "fp8x"
    # fp8 x: [KP][128, 2, T]; bf16 x: [KD][128, T]
    xq_d = (nc.dram_tensor("xq", [KP, P, 2, T], FP8, kind="ExternalInput")
            if fp8_gates else None)
    xb_d = (nc.dram_tensor("xb", [KD, P, T], BF16, kind="ExternalInput")
            if use_xb else None)
    xh_d = (nc.dram_tensor("xh", [KD, P, T], FP8, kind="ExternalInput")
            if mm_mode == "fp8x" else None)
    wq_d = (nc.dram_tensor("wq", [fp8_gates, KP, P, 2, H], FP8,
                           kind="ExternalInput") if fp8_gates else None)
    wb_d = (nc.dram_tensor("wb", [bf_gates, KD, P, H], BF16,
                           kind="ExternalInput") if bf_gates else None)
    # 6 bias groups packed per partition plus a -0.5 constant column:
    # [b_f | b_i | b_h | b_h + 0.5 | -b_f | -b_i | -0.5]
    bias_d = nc.dram_tensor("biasp", [P, 6 * HB + 1], F32,
                            kind="ExternalInput")
    h0_d = nc.dram_tensor("h0p", [P, HB], F32, kind="ExternalInput")
    ht_d = nc.dram_tensor("ht", [H, T], out_dt, kind="ExternalOutput")

    NTE = T // tte
    NH = tte // 512   # matmul N=512 groups per tile
    n_zbufs = 8 * 512 // tte  # PSUM: 8 banks, each z tile is tte/512 banks

    with tile.TileContext(nc) as tc:
        with (
            tc.tile_pool(name="xp", bufs=1) as xp,
            tc.tile_pool(name="wp", bufs=1) as wp,
            tc.tile_pool(name="cp", bufs=1) as cp,
            tc.tile_pool(name="ps", bufs=n_zbufs, space="PSUM") as ps,
            tc.tile_pool(name="sb", bufs=sb_bufs) as sb,
            tc.tile_pool(name="hop", bufs=hop_bufs) as hop,
        ):
            # tiny constants first, then prime the ACT sigmoid table so the
            # ~1.3us table load is off the critical path
            bias = cp.tile([P, 6 * HB + 1], F32, tag="bias")
            nc.sync.dma_start(bias[:], bias_d.ap())
            h0 = cp.tile([P, HB], F32, tag="h0")
            nc.sync.dma_start(h0[:], h0_d.ap())
            warm = cp.tile([P, 1], F32, tag="warm")
            nc.scalar.activation(warm[:], h0[:, 0:1], AF.Sigmoid)

            xq = [xp.tile([P, 2, T], FP8, tag=f"xq{p}", name=f"xq{p}")
                  for p in range(KP)] if fp8_gates else []
            xb = [xp.tile([P, T], BF16, tag=f"xb{k}", name=f"xb{k}")
                  for k in range(KD)] if use_xb else []
            xh = [xp.tile([P, T], FP8, tag=f"xh{k}", name=f"xh{k}")
                  for k in range(KD)] if mm_mode == "fp8x" else []
            wq = [[wp.tile([P, 2, H], FP8, tag=f"wq{g}{p}", name=f"wq{g}{p}")
                   for p in range(KP)] for g in range(fp8_gates)]
            wb = [[wp.tile([P, H], BF16, tag=f"wb{g}{k}", name=f"wb{g}{k}")
                   for k in range(KD)] for g in range(bf_gates)]

            def load_w():
                for g in range(fp8_gates):
                    for p in range(KP):
                        nc.sync.dma_start(wq[g][p][:], wq_d.ap()[g, p])
                for g in range(bf_gates):
                    for k in range(KD):
                        nc.sync.dma_start(wb[g][k][:], wb_d.ap()[g, k])

            def load_x(tt):
                tsl = slice(tt * tte, (tt + 1) * tte)
                if fp8_gates:
                    for p in range(KP):
                        nc.sync.dma_start(xq[p][:, :, tsl],
                                          xq_d.ap()[p][:, :, tsl])
                if use_xb:
                    for k in range(KD):
                        nc.sync.dma_start(xb[k][:, tsl], xb_d.ap()[k][:, tsl])
                if mm_mode == "fp8x":
                    for k in range(KD):
                        nc.sync.dma_start(xh[k][:, tsl], xh_d.ap()[k][:, tsl])

            load_w()
            if not (loop_reps and loads_in_loop):
                for tt in range(NTE):
                    load_x(tt)

            import contextlib
            loop_cm = (tc.For_i(0, loop_reps, 1) if loop_reps
                       else contextlib.nullcontext())
            with loop_cm:
             for _rep in range(reps):
              if loop_reps and loads_in_loop:
                  for tt in range(NTE):
                      load_x(tt)
              prev = [None] * HB
              pend = None

              def front(tt, hb):
                    tsl = slice(tt * tte, (tt + 1) * tte)
                    hsl = slice(hb * P, (hb + 1) * P)
                    bfv = bias[:, 0 * HB + hb:0 * HB + hb + 1]
                    biv = bias[:, 1 * HB + hb:1 * HB + hb + 1]
                    bh05 = bias[:, 3 * HB + hb:3 * HB + hb + 1]
                    bhv = bias[:, 2 * HB + hb:2 * HB + hb + 1]
                    zf = ps.tile([P, tte], F32, tag="z")
                    zi = ps.tile([P, tte], F32, tag="z")
                    zh = ps.tile([P, tte], F32, tag="z")
                    zs = (zf, zi, zh)
                    if "mm" not in ablate:
                        # Interleave the h gate's plain matmuls between the
                        # f/i DoubleRow groups: a DR matmul's 256-row weight
                        # load hides behind the preceding plain matmul's
                        # stream (pure-DR streams stall on ldweights).
                        def h_mms(nh, ks):
                            z = zs[fp8_gates]
                            c0 = tt * tte + nh * 512
                            for k in ks:
                                rhs = (xb[k][:, c0:c0 + 512] if use_xb
                                       else xh[k][:, c0:c0 + 512])
                                nc.tensor.matmul(
                                    z[:, nh * 512:(nh + 1) * 512],
                                    wb[0][k][:, hsl],
                                    rhs,
                                    start=(k == 0), stop=(k == KD - 1),
                                )

                        def dr_mms(g, nh):
                            z = zs[g]
                            c0 = tt * tte + nh * 512
                            for p in range(KP):
                                nc.tensor.matmul(
                                    z[:, nh * 512:(nh + 1) * 512],
                                    wq[g][p][:, :, hsl],
                                    xq[p][:, :, c0:c0 + 512],
                                    start=(p == 0), stop=(p == KP - 1),
                                    perf_mode=MM.DoubleRow,
                                )

                        def dr_one(g, nh, p):
                            z = zs[g]
                            c0 = tt * tte + nh * 512
                            nc.tensor.matmul(
                                z[:, nh * 512:(nh + 1) * 512],
                                wq[g][p][:, :, hsl],
                                xq[p][:, :, c0:c0 + 512],
                                start=(p == 0), stop=(p == KP - 1),
                                perf_mode=MM.DoubleRow,
                            )

                        if bf_gates:
                            # alternate plain h matmuls with single DR
                            # matmuls so every DR 256-row weight load hides
                            # behind ~1024 cycles of streaming
                            for nh in range(NH):
                                h_mms(nh, [0])
                                dr_one(0, nh, 0)
                                h_mms(nh, [1])
                                dr_one(0, nh, 1)
                                h_mms(nh, [2])
                                dr_one(1, nh, 0)
                                h_mms(nh, [3])
                                dr_one(1, nh, 1)
                        else:
                            for nh in range(NH):
                                for g in range(fp8_gates):
                                    dr_mms(g, nh)
                    fsc = 1.0 / W8SCALE if fp8_gates >= 1 else 1.0
                    isc = 1.0 / W8SCALE if fp8_gates >= 2 else 1.0
                    hsc = 1.0 / W8SCALE if fp8_gates >= 3 else 1.0
                    tf = sb.tile([P, tte], ew_dt, tag="tf")
                    nc.scalar.activation(tf[:], zf[:], AF.Sigmoid,
                                         bias=bfv, scale=fsc)
                    ti = sb.tile([P, tte], ew_dt, tag="ti")
                    nc.scalar.activation(ti[:], zi[:], AF.Sigmoid,
                                         bias=biv, scale=isc)
                    gg = sb.tile([P, tte], ew_dt, tag="gg")
                    if gg_mode == "lin":
                        # lin = zh + (b_h + 0.5) via ACT Identity consumes
                        # zh's PSUM immediately; th = sigmoid(lin - 0.5)
                        # reads SBUF instead of PSUM.
                        lin = sb.tile([P, tte], ew_dt, tag="lin")
                        nc.scalar.activation(lin[:], zh[:], AF.Identity,
                                             bias=bh05, scale=hsc)
                        th = sb.tile([P, tte], ew_dt, tag="th")
                        nc.scalar.activation(th[:], lin[:], AF.Sigmoid,
                                             bias=bias[:, 6 * HB:6 * HB + 1])
                        nc.vector.tensor_tensor(gg[:], lin[:], th[:], ALU.max)
                    else:
                        assert hsc == 1.0
                        th = sb.tile([P, tte], ew_dt, tag="th")
                        nc.scalar.activation(th[:], zh[:], AF.Sigmoid,
                                             bias=bhv)
                        nc.vector.scalar_tensor_tensor(
                            gg[:], zh[:], bh05, th[:], ALU.add, ALU.max)
                    den32 = sb.tile([P, tte], F32, tag="den32")
                    if den_eng == "dma":
                        # den = tf + ti on the DMA engines: gpsimd-initiated
                        # DMAs can cast (fp16->f32) and accumulate, freeing
                        # Pool/DVE cycles entirely.
                        nc.gpsimd.dma_start(den32[:], tf[:])
                        nc.gpsimd.dma_start(den32[:], ti[:],
                                            accum_op=ALU.add)
                    else:
                        getattr(nc, den_eng).tensor_tensor(den32[:], tf[:],
                                                           ti[:], ALU.add)
                    return dict(tt=tt, hb=hb, ti=ti, gg=gg, den32=den32)

              def back(st):
                    tt, hb = st["tt"], st["hb"]
                    tsl = slice(tt * tte, (tt + 1) * tte)
                    hsl = slice(hb * P, (hb + 1) * P)
                    rr = sb.tile([P, tte], F32, tag="rr")
                    nc.vector.reciprocal_approx_fast(rr[:], st["den32"][:])
                    qq = sb.tile([P, tte], ew_dt, tag="qq")
                    if qq_eng == "dma":
                        nc.gpsimd.dma_start(qq[:], st["ti"][:])
                        nc.gpsimd.dma_start(qq[:], rr[:], accum_op=ALU.mult)
                    else:
                        getattr(nc, qq_eng).tensor_tensor(qq[:], st["ti"][:],
                                                          rr[:], ALU.mult)
                    aa = sb.tile([P, tte], ew_dt, tag="aa")
                    nc.vector.tensor_scalar(aa[:], qq[:], -1.0, 1.0,
                                            ALU.mult, ALU.add)
                    bb = sb.tile([P, tte], ew_dt, tag="bb")
                    getattr(nc, bb_eng).tensor_tensor(bb[:], qq[:],
                                                      st["gg"][:], ALU.mult)
                    if "scan" not in ablate:
                        ho = hop.tile([P, tte], out_dt, tag="ho")
                        init = (h0[:, hb:hb + 1] if tt == 0
                                else prev[hb][:, tte - 1:tte])
                        for j0 in range(0, tte, scan_chunk):
                            jsl = slice(j0, j0 + scan_chunk)
                            nc.vector.tensor_tensor_scan(
                                ho[:, jsl], aa[:, jsl], bb[:, jsl], init,
                                ALU.mult, ALU.add)
                            init = ho[:, j0 + scan_chunk - 1:j0 + scan_chunk]
                        prev[hb] = ho
                    else:
                        ho = bb
                    if "dma" not in ablate:
                        nc.sync.dma_start(ht_d.ap()[hsl, tsl], ho[:])

              for tt in range(NTE):
                for hb in range(HB):
                    st = front(tt, hb)
                    if pend is not None:
                        back(pend)
                    pend = st
              back(pend)
            if "dma" in ablate or "act" in ablate:
                dummy = cp.tile([P, HB], out_dt, tag="dummy")
                nc.vector.memset(dummy[:], 0.0)
                nc.sync.dma_start(ht_d.ap()[0:P, 0:HB], dummy[:])

    nc.compile()
    return nc


def _get_module():
    if "nc" not in _CACHE:
        _CACHE["nc"] = _build()
    return _CACHE["nc"]


class _Runner:
    """Caches a compiled 8-core shard_map'd PJRT executable of the Bass
    module so repeat kernel() calls skip jax retracing/compilation."""

    def __init__(self, nc):
        import jax
        from jax.experimental.shard_map import shard_map
        from jax.sharding import Mesh, PartitionSpec

        from concourse import bass2jax

        bass2jax.install_neuronx_cc_hook()
        self.nc = nc
        partition_name = (nc.partition_id_tensor.name
                          if nc.partition_id_tensor else None)
        in_names, out_names, out_avals = [], [], []
        for alloc in nc.m.functions[0].allocations:
            if not isinstance(alloc, mybir.MemoryLocationSet):
                continue
            name = alloc.memorylocations[0].name
            if alloc.kind == "ExternalInput":
                if name != partition_name:
                    in_names.append(name)
            elif alloc.kind == "ExternalOutput":
                out_names.append(name)
                out_avals.append(jax.core.ShapedArray(
                    tuple(alloc.tensor_shape), mybir.dt.np(alloc.dtype)))
        self.in_names = in_names
        self.out_names = out_names
        self.out_avals = out_avals
        n_params, n_outs = len(in_names), len(out_names)
        all_names = list(in_names) + list(out_names)
        if partition_name is not None:
            all_names.append(partition_name)

        def _body(*args):
            operands = list(args)
            if partition_name is not None:
                operands.append(bass2jax.partition_id_tensor())
            return tuple(bass2jax._bass_exec_p.bind(
                *operands,
                out_avals=tuple(out_avals),
                in_names=tuple(all_names),
                out_names=tuple(out_names),
                lowering_input_output_aliases=(),
                sim_require_finite=True,
                sim_require_nnan=True,
                nc=nc,
            ))

        devices = jax.devices()[:B]
        mesh = Mesh(np.asarray(devices), ("core",))
        specs = (PartitionSpec("core"),) * (n_params + n_outs)
        out_specs = (PartitionSpec("core"),) * n_outs
        donate = tuple(range(n_params, n_params + n_outs))
        self._jitted = jax.jit(
            shard_map(_body, mesh=mesh, in_specs=specs,
                      out_specs=out_specs, check_rep=False),
            donate_argnums=donate, keep_unused=True)
        self._compiled = None

    def concat_args(self, in_maps):
        concat_in = [
            np.concatenate([np.asarray(m[name]) for m in in_maps], axis=0)
            for name in self.in_names
        ]
        concat_zeros = [
            np.zeros((B * a.shape[0], *a.shape[1:]), a.dtype)
            for a in self.out_avals
        ]
        return concat_in + concat_zeros

    def compiled(self, args):
        if self._compiled is None:
            self._compiled = self._jitted.lower(*args).compile()
        return self._compiled

    def __call__(self, in_maps):
        import jax
        args = self.concat_args(in_maps)
        outs = jax.block_until_ready(self.compiled(args)(*args))
        return [
            {name: np.asarray(outs[i]).reshape(B, *self.out_avals[i].shape)[c]
             for i, name in enumerate(self.out_names)}
            for c in range(B)
        ]


def _get_runner():
    if "runner" not in _CACHE:
        _CACHE["runner"] = _Runner(_get_module())
    return _CACHE["runner"]


def make_in_maps(x, h_0, W_f, b_f, W_i, b_i, W_h, b_h,
                 mm_mode="fp8x"):
    x = np.asarray(x, np.float32)
    h_0 = np.asarray(h_0, np.float32)
    Ws = [np.asarray(W_f, np.float32), np.asarray(W_i, np.float32),
          np.asarray(W_h, np.float32)]
    fp8_gates = 2 if mm_mode in ("mix", "fp8x") else \
        (3 if mm_mode == "fp8" else 0)
    bf_gates = 3 - fp8_gates
    use_xb = bf_gates and mm_mode != "fp8x"

    # fp8 weights: [g][kp][r, j, h] = 32*W_g[h, (2p+j)*128+r], stored
    # [fp8_gates, KP, P, 2*H]
    if fp8_gates:
        wq = np.empty((fp8_gates, KP, P, 2, H), dtype=ml_dtypes.float8_e4m3)
        for g in range(fp8_gates):
            Wt = (Ws[g] * W8SCALE).T.reshape(KP, 2, P, H)  # [kp, j, r, h]
            wq[g] = Wt.transpose(0, 2, 1, 3).astype(ml_dtypes.float8_e4m3)
        wq = np.ascontiguousarray(wq)
    # bf16 weights (the last bf_gates of [f,i,h]): [g][k][r, h] = W[h, k*128+r]
    if bf_gates:
        wb = np.empty((bf_gates, KD, P, H), dtype=ml_dtypes.bfloat16)
        for g in range(bf_gates):
            Wt = Ws[fp8_gates + g].T.reshape(KD, P, H)
            wb[g] = Wt.astype(ml_dtypes.bfloat16)
        wb = np.ascontiguousarray(wb)

    b_h = np.asarray(b_h, np.float32)
    b_f = np.asarray(b_f, np.float32)
    b_i = np.asarray(b_i, np.float32)
    biasp = (np.stack([b_f, b_i, b_h, b_h + 0.5, -b_f, -b_i])
             .astype(np.float32).reshape(6, HB, P).transpose(2, 0, 1)
             .reshape(P, 6 * HB))
    biasp = np.ascontiguousarray(
        np.concatenate([biasp, np.full((P, 1), -0.5, np.float32)], axis=1))

    in_maps = []
    for b in range(B):
        m = {"biasp": biasp,
             "h0p": np.ascontiguousarray(h_0[b].reshape(HB, P).T)}
        xT = x[b].T  # [D, T]
        if fp8_gates:
            x8 = xT.astype(ml_dtypes.float8_e4m3)  # [D, T] fp8
            xq = x8.reshape(KP, 2, P, T).transpose(0, 2, 1, 3)
            m["xq"] = np.ascontiguousarray(xq)
            m["wq"] = wq
            if mm_mode == "fp8x":
                m["xh"] = np.ascontiguousarray(x8.reshape(KD, P, T))
        if bf_gates:
            if use_xb:
                m["xb"] = np.ascontiguousarray(
                    xT.reshape(KD, P, T).astype(ml_dtypes.bfloat16))
            m["wb"] = wb
        in_maps.append(m)
    return in_maps


def kernel(x, h_0, W_f, b_f, W_i, b_i, W_h, b_h):
    in_maps = make_in_maps(x, h_0, W_f, b_f, W_i, b_i, W_h, b_h)
    results = _get_runner()(in_maps)
    out = np.empty((B, T, H), np.float32)
    for b in range(B):
        out[b] = results[b]["ht"].astype(np.float32).T
    return out
